# revision 7
# baseline (speedup 1.0000x reference)
"""Trainium2 Bass kernel for a custom LSTM cell.

reference:
    z = concat([h_tm1, inputs], -1) @ kernel      # [B, 4U]
    i, f, g, o = split(z, 4, -1)
    c = sigmoid(f) * c_tm1 + sigmoid(i) * tanh(g)
    h = sigmoid(o) * tanh(c)
    returns (h, c)

Sharding over 8 NeuronCores: 2-way over batch x 4-way over units
(each gate's block co-located per core).  Per core:
    z_blk = A_half @ W[:, 4 gate slices of 256] via fp32r matmuls
    (TF32-like, full PE rate), gate math on-chip, outputs [1024, 256]
    h/c blocks.  Host only slices/concatenates.

Schedule (per core):
  phase 1: k-outer round-robin over 8 open PSUM groups = all 8 batch
           sub-tiles x (i|f) columns, consuming at/wk chunks in DMA
           arrival order so the PE stays busy and HAM-warm during the
           load window.  Each group closes with one Sigmoid -> sig_if,
           freeing its PSUM bank.
  phase 2: per-m serial accumulation of (g|o) columns from SBUF-resident
           data, epilogue (tanh/sigmoid/DVE combine) pipelined per m.
Inputs stream on the Sync queue in consumption order; outputs go out on
the GpSimd queue.
"""

import sys

sys.path.insert(0, "/opt/trn_rl_repo")

import numpy as np

BATCH, INPUT_DIM, UNITS = 2048, 512, 1024
K = UNITS + INPUT_DIM  # contraction dim, 1536
R, C = 2, 4  # batch halves x unit quarters
BR = BATCH // R  # 1024 batch rows per core
UC = UNITS // C  # 256 units per core
KS = K // 128  # 12 k-subtiles
M = BR // 128  # 8 batch sub-chunks per core

_CACHE = {}


def _build_nc():
    import concourse.tile as tile
    from concourse import bacc, mybir

    f32 = mybir.dt.float32
    f32r = mybir.dt.float32r
    Sig = mybir.ActivationFunctionType.Sigmoid
    Tanh = mybir.ActivationFunctionType.Tanh

    nc = bacc.Bacc("TRN2")
    at_in = nc.declare_dram_parameter("at", [K, BR], f32r, isOutput=False)
    wk_in = nc.declare_dram_parameter("wk", [K, 4 * UC], f32r, isOutput=False)
    ct_in = nc.declare_dram_parameter("ct", [BR, UC], f32, isOutput=False)
    h_out = nc.declare_dram_parameter("h_out", [BR, UC], f32, isOutput=True)
    c_out = nc.declare_dram_parameter("c_out", [BR, UC], f32, isOutput=True)

    with tile.TileContext(nc) as tc:
        with (
            tc.tile_pool(name="data", bufs=1) as data,
            tc.tile_pool(name="work", bufs=3) as work,
            tc.tile_pool(name="psum", bufs=8, space="PSUM") as psum,
        ):
            at = data.tile([128, KS, BR], f32r)
            wk_lo = data.tile([128, KS, 512], f32r)  # i|f columns
            wk_hi = data.tile([128, KS, 512], f32r)  # g|o columns
            ct = data.tile([128, M, UC], f32)
            sig_if = data.tile([128, M, 512], f32)
            fc_all = data.tile([128, M, UC], f32)

            at_r = at_in[:].rearrange("(ko p) n -> p ko n", p=128)
            wklo_r = wk_in[:, 0:512].rearrange("(ko p) n -> p ko n", p=128)
            wkhi_r = wk_in[:, 512:1024].rearrange("(ko p) n -> p ko n", p=128)
            ct_r = ct_in[:].rearrange("(m p) u -> p m u", p=128)

            # inputs in consumption order; at on Sync, wk_lo on Vector so the
            # serial ~0.7us descriptor-gen streams run in parallel.  First two
            # chunks are single-k so the PE can start as early as possible.
            chunks = [slice(0, 1), slice(1, 2)] + [
                slice(2 * j, 2 * j + 2) for j in range(1, KS // 2)
            ]
            for ks in chunks:
                nc.sync.dma_start(at[:, ks, :], at_r[:, ks, :])
            for ks in chunks:
                nc.scalar.dma_start(wk_lo[:, ks, :], wklo_r[:, ks, :])
            for j in range(KS // 2):
                ks = slice(2 * j, 2 * j + 2)
                nc.sync.dma_start(wk_hi[:, ks, :], wkhi_r[:, ks, :])
            for j in range(M // 2):
                ms2 = slice(2 * j, 2 * j + 2)
                nc.sync.dma_start(ct[:, ms2, :], ct_r[:, ms2, :])

            # phase 1: all m, i|f columns, k-outer round-robin
            plo = [
                psum.tile([128, 512], f32, tag="ps", name=f"plo{m}") for m in range(M)
            ]
            for k in range(KS):
                for m in range(M):
                    nc.tensor.matmul(
                        plo[m][:],
                        at[:, k, m * 128 : (m + 1) * 128],
                        wk_lo[:, k, :],
                        start=(k == 0),
                        stop=(k == KS - 1),
                    )
            for m in range(M):
                nc.scalar.activation(sig_if[:, m, :], plo[m][:], Sig)
            # f * c_tm1 off the epilogue critical path (DVE is idle here)
            for m in range(M):
                nc.vector.tensor_mul(
                    fc_all[:, m, :], sig_if[:, m, UC : 2 * UC], ct[:, m, :]
                )

            # phase 2: per-m serial g|o accumulation + epilogue
            for m in range(M):
                ms = slice(m * 128, (m + 1) * 128)
                phi = psum.tile([128, 512], f32, tag="ps", name=f"phi{m}")
                for k in range(KS):
                    nc.tensor.matmul(
                        phi[:],
                        at[:, k, ms],
                        wk_hi[:, k, :],
                        start=(k == 0),
                        stop=(k == KS - 1),
                    )
                tg = work.tile([128, UC], f32, tag="tg")
                nc.scalar.activation(tg[:], phi[:, 0:UC], Tanh)
                so = work.tile([128, UC], f32, tag="so")
                nc.scalar.activation(so[:], phi[:, UC : 2 * UC], Sig)
                ig = work.tile([128, UC], f32, tag="ig")
                nc.vector.tensor_mul(ig[:], sig_if[:, m, 0:UC], tg[:])
                cn = work.tile([128, UC], f32, tag="cn")
                nc.vector.tensor_add(cn[:], fc_all[:, m, :], ig[:])
                th = work.tile([128, UC], f32, tag="th")
                nc.scalar.activation(th[:], cn[:], Tanh)
                hn = work.tile([128, UC], f32, tag="hn")
                nc.vector.tensor_mul(hn[:], so[:], th[:])
                nc.gpsimd.dma_start(c_out[ms, :], cn[:])
                nc.gpsimd.dma_start(h_out[ms, :], hn[:])

    nc.compile()
    return nc


def get_nc():
    if "nc" not in _CACHE:
        _CACHE["nc"] = _build_nc()
    return _CACHE["nc"]


def make_in_maps(inputs, h_tm1, c_tm1, kernel):
    x = np.ascontiguousarray(np.asarray(inputs, dtype=np.float32))
    h = np.ascontiguousarray(np.asarray(h_tm1, dtype=np.float32))
    c = np.ascontiguousarray(np.asarray(c_tm1, dtype=np.float32))
    w = np.ascontiguousarray(np.asarray(kernel, dtype=np.float32))
    at_full = np.ascontiguousarray(np.concatenate([h, x], axis=1).T)  # [K, B]
    in_maps = []
    for core in range(R * C):
        r, ci = divmod(core, C)
        at_np = np.ascontiguousarray(at_full[:, r * BR : (r + 1) * BR])
        wk_np = np.ascontiguousarray(
            np.concatenate(
                [w[:, g * UNITS + ci * UC : g * UNITS + (ci + 1) * UC] for g in range(4)],
                axis=1,
            )
        )
        ct_np = np.ascontiguousarray(c[r * BR : (r + 1) * BR, ci * UC : (ci + 1) * UC])
        in_maps.append({"at": at_np, "wk": wk_np, "ct": ct_np})
    return in_maps


def assemble(results):
    h_new = np.empty((BATCH, UNITS), dtype=np.float32)
    c_new = np.empty((BATCH, UNITS), dtype=np.float32)
    for core in range(R * C):
        r, ci = divmod(core, C)
        h_new[r * BR : (r + 1) * BR, ci * UC : (ci + 1) * UC] = results[core]["h_out"]
        c_new[r * BR : (r + 1) * BR, ci * UC : (ci + 1) * UC] = results[core]["c_out"]
    return h_new, c_new


def kernel(inputs, h_tm1, c_tm1, kernel):
    from concourse.bass_utils import run_bass_kernel_spmd

    nc = get_nc()
    in_maps = make_in_maps(inputs, h_tm1, c_tm1, kernel)
    res = run_bass_kernel_spmd(nc, in_maps, list(range(R * C)), trace=False)
    return assemble(res.results)
